# revision 23
# baseline (speedup 1.0000x reference)
"""AdaptiveGraphLearning forward on 8 Trainium2 NeuronCores.

Data-parallel over batch B=64: each core processes 8 batches; the (N,N)
adjacency parameter and tiny edge-MLP weights are replicated (the forward
pass needs no collectives).

Per-core dataflow (8 batches as 4 pairs):
  - HWDGE-DMA two batches of temporal_features per transfer as
    (128, 8192) f32 tiles: partition p=(b_lo,n), free=(h,t) -- 32-64KB
    contiguous DRAM per partition for near-peak HBM bandwidth. Pair 0 is
    split into 2MB quarters spread over both HWDGE rings (SP + ACT) to
    shorten the pipeline-fill latency; later pairs ride the SP ring as
    4MB halves while ACT is busy with evacuations.
  - Sum over t: one f32->bf16 VectorE fold, then bf16 folds in 2x mode
    (t:128->64->32->16) and a short 1x reduce -> R (128=(b_lo,n), 128=h).
  - TensorE transpose -> node_T bf16 (h on partitions). 1/T folded into
    the W1 halves host-side.
  - Edge MLP entirely in bf16 matmuls (1 cycle/row):
      X_pre = Wi.T @ node_T[:,i(bcast)] + Wj.T @ node_T[:,j(bcast)] (PSUM)
      X  = relu(X_pre + b1)   ScalarE evacuation -> bf16 SBUF
      h2 = relu(W2.T X + b2)  evacuations mostly ScalarE, 1-in-8 VectorE
      F  = W3blk.T h2         one-hot block weights accumulate (8,512)
  - F(8,512) -> F(64,64) via SBUF->SBUF DMA (identical linearized element
    order), F^T on TensorE, then
      out = (relu(G + F + F^T) + I) / row-sum
    with G = 0.25*(ap+ap^T) precomputed host-side and the 0.25
    symmetrization factor folded into W3.

Harness notes: walrus in this container accepts a single semaphore wait
per instruction, so a BIR-level pass splits Tile's multi-wait
instructions onto EventSemaphore carriers; the Tile kernel-tail drain
gets the same treatment at build time.
"""
import sys

sys.path.insert(0, '/opt/trn_rl_repo')

import numpy as np

B, N, H, T = 64, 64, 128, 128
NCORES = 8
B_LOC = B // NCORES      # 8 batches per core
PAIRS = B_LOC // 2       # 4 batch pairs per core
NCH = N // 8             # 8 i-chunks per batch (8 i x 64 j = 512 wide)

_CACHE = {}


def _install_wait_splitter():
    """walrus's per-instruction sync structs hold a single semaphore wait;
    Tile can emit several on one instruction. Split extras onto preceding
    single-wait Drain instructions at the BIR-JSON level."""
    if _CACHE.get('wait_splitter'):
        return
    import json

    import concourse.bass2jax as bass2jax

    orig = bass2jax.compile_bir_kernel

    def split_waits_in_bir(bir_bytes):
        d = json.loads(bir_bytes)
        n_new = [0]
        for fn in d.get("functions", []):
            for bb in fn.get("blocks", []):
                out = []
                for ins in bb.get("instructions", []):
                    si = ins.get("sync_info") or {}
                    waits = si.get("on_wait") or []
                    if len(waits) > 1:
                        for w in waits[:-1]:
                            n_new[0] += 1
                            out.append({
                                "engine": ins["engine"],
                                "ins": [],
                                "name": f"IWS-{n_new[0]}",
                                "opcode": "EventSemaphore",
                                "outs": [],
                                "sync_info": {"on_update": [], "on_wait": [w]},
                            })
                        si["on_wait"] = [waits[-1]]
                    out.append(ins)
                bb["instructions"] = out
        return json.dumps(d).encode()

    def wrapper(ant_bir_str, *a, **kw):
        return orig(split_waits_in_bir(ant_bir_str), *a, **kw)

    bass2jax.compile_bir_kernel = wrapper
    _CACHE['wait_splitter'] = True


def _split_drain_tile_context(tile_mod, mybir, nc):
    """TileContext whose kernel-tail drain splits its semaphore waits across
    sequential Drain instructions (walrus CTRL insts accept one wait)."""
    from concourse.tile import ScopedClock

    class SplitDrainTileContext(tile_mod.TileContext):
        def _drain_and_barrier(self, tick_clock, wait_clock):
            drain_inst = self.nc.sync.drain()
            wait_clock.add_sem_waits(
                drain_inst.ins, ScopedClock({None: tick_clock.global_clock})
            )
            waits = list(drain_inst.ins.sync_info.on_wait)
            if len(waits) > 1:
                drain_inst.ins.sync_info = mybir.SyncInfo(
                    on_wait=waits[:1],
                    on_update=list(drain_inst.ins.sync_info.on_update),
                )
                for i in range(1, len(waits)):
                    extra = self.nc.sync.drain()
                    extra.ins.sync_info = mybir.SyncInfo(
                        on_wait=waits[i : i + 1], on_update=[]
                    )
            self.nc.all_engine_barrier()
            assert self.sems is not None
            popped = self.nc._tile_sem_poison_stack.pop()
            assert popped is self._sem_poison
            self.nc.clear_and_free_semaphores(list(self.sems.allocated().values()))
            self.nc.all_engine_barrier()

    return SplitDrainTileContext(nc)


def build_nc():
    import concourse.bass as bass
    import concourse.tile as tile
    from concourse import mybir
    from contextlib import ExitStack

    f32 = mybir.dt.float32
    bf16 = mybir.dt.bfloat16
    AF = mybir.ActivationFunctionType
    ALU = mybir.AluOpType
    AX = mybir.AxisListType

    nc = bass.Bass()
    tf = nc.declare_dram_parameter("tf", [B_LOC, N, H, T], f32, isOutput=False)
    Wi = nc.declare_dram_parameter("Wi", [H, H], bf16, isOutput=False)
    Wj = nc.declare_dram_parameter("Wj", [H, H], bf16, isOutput=False)
    b1c = nc.declare_dram_parameter("b1c", [H, 1], f32, isOutput=False)
    W2 = nc.declare_dram_parameter("W2", [H, H // 2], bf16, isOutput=False)
    b2c = nc.declare_dram_parameter("b2c", [H // 2, 1], f32, isOutput=False)
    W3blk = nc.declare_dram_parameter("W3blk", [H // 2, 8 * NCH], bf16, isOutput=False)
    b3c = nc.declare_dram_parameter("b3c", [8, 1], f32, isOutput=False)
    G = nc.declare_dram_parameter("G", [N, N], f32, isOutput=False)
    I64 = nc.declare_dram_parameter("I64", [N, N], f32, isOutput=False)
    I128 = nc.declare_dram_parameter("I128", [H, H], f32, isOutput=False)
    out_ext = nc.declare_dram_parameter("out", [B_LOC, N, N], f32, isOutput=True)

    MONO = _CACHE.get('cfg_mono', False)
    NOBIAS = _CACHE.get('cfg_nobias', False)

    with _split_drain_tile_context(tile, mybir, nc) as tc, ExitStack() as ctx:
        consts = ctx.enter_context(tc.tile_pool(name="consts", bufs=1))
        tf_pool = ctx.enter_context(
            tc.tile_pool(name="tf", bufs=2 if MONO else 4))
        fold_pool = ctx.enter_context(tc.tile_pool(name="fold", bufs=2))
        red_pool = ctx.enter_context(tc.tile_pool(name="red", bufs=2))
        x_pool = ctx.enter_context(tc.tile_pool(name="x", bufs=4))
        h2_pool = ctx.enter_context(tc.tile_pool(name="h2", bufs=4))
        ff_pool = ctx.enter_context(tc.tile_pool(name="ff", bufs=2))
        ep_pool = ctx.enter_context(tc.tile_pool(name="ep", bufs=2))
        ps_x = ctx.enter_context(tc.tile_pool(name="ps_x", bufs=2, space="PSUM"))
        ps_h2 = ctx.enter_context(tc.tile_pool(name="ps_h2", bufs=2, space="PSUM"))
        ps_w3 = ctx.enter_context(tc.tile_pool(name="ps_w3", bufs=2, space="PSUM"))
        ps_t = ctx.enter_context(tc.tile_pool(name="ps_t", bufs=1, space="PSUM"))
        ps_ft = ctx.enter_context(tc.tile_pool(name="ps_ft", bufs=1, space="PSUM"))
        if True:
            wi_sb = consts.tile([H, H], bf16)
            nc.scalar.dma_start(wi_sb[:], Wi[:])
            wj_sb = consts.tile([H, H], bf16)
            nc.scalar.dma_start(wj_sb[:], Wj[:])
            w2_sb = consts.tile([H, H // 2], bf16)
            nc.scalar.dma_start(w2_sb[:], W2[:])
            w3_sb = consts.tile([H // 2, 8 * NCH], bf16)
            nc.scalar.dma_start(w3_sb[:], W3blk[:])
            b1_sb = consts.tile([H, 1], f32)
            nc.scalar.dma_start(b1_sb[:], b1c[:])
            b2_sb = consts.tile([H // 2, 1], f32)
            nc.scalar.dma_start(b2_sb[:], b2c[:])
            b3_sb = consts.tile([8, 1], f32)
            nc.scalar.dma_start(b3_sb[:], b3c[:])
            g_sb = consts.tile([N, N], f32)
            nc.scalar.dma_start(g_sb[:], G[:])
            i64_sb = consts.tile([N, N], f32)
            nc.scalar.dma_start(i64_sb[:], I64[:])
            i128_sb = consts.tile([H, H], f32)
            nc.scalar.dma_start(i128_sb[:], I128[:])

            HQ = H // 2

            def load_pair(c):
                if MONO:
                    # one monolithic 8MB transfer: single-queue DMA reaches
                    # ~425 GB/s only for large transfers (4MB: ~334)
                    tft = tf_pool.tile([128, H, T], f32, name=f"tft{c}",
                                       tag="tft")
                    nc.sync.dma_start(tft[:], tf[2 * c : 2 * c + 2, :, :, :])
                    return [tft[:, 0:HQ, :], tft[:, HQ:H, :]]
                halves = []
                for hh in range(2):
                    tft = tf_pool.tile([128, HQ, T], f32, name=f"tft{c}_{hh}",
                                       tag="tft")
                    # For the first two pairs, the odd halves ride the ACT
                    # HWDGE ring (its trigger sits at the head of ACT's
                    # still-idle queue) -> both rings run concurrently during
                    # the pipeline-fill window, ~2x load bandwidth.
                    eng = nc.scalar if (c < 3 and hh == 1) else nc.sync
                    eng.dma_start(
                        tft[:],
                        tf[2 * c : 2 * c + 2, :, hh * HQ : (hh + 1) * HQ, :])
                    halves.append(tft[:])
                return halves

            def load_pair0_quarters():
                # Pair 0 in 2MB quarters across both rings with a dedicated
                # pool tag: the first fold starts right after the first
                # quarter lands, and later pairs' half-tiles don't contend
                # for these slots.
                qs = []
                HF = H // 4
                for q in range(4):
                    tft = tf_pool.tile([128, HF, T], f32, name=f"tfq{q}",
                                       tag="tft")
                    eng = nc.sync if q % 2 == 0 else nc.scalar
                    eng.dma_start(
                        tft[:], tf[0:2, :, q * HF : (q + 1) * HF, :])
                    qs.append(tft[:])
                return qs

            def emit_folds(c, parts):
                # Sum over T: R[p=(b_lo,n), h] = sum_t tf[2c+b_lo, n, h, t].
                # Emitted BEFORE the previous pair's MLP so these sit ahead
                # of the wait-gated epilogue ops in the DVE FIFO and start
                # the moment their data lands.
                r_sb = red_pool.tile([128, H], f32, tag="r", name=f"r{c}")
                npc = len(parts)
                HP = H // npc
                for hh in range(npc):
                    th = parts[hh]
                    f1 = fold_pool.tile([128, HP, 64], bf16, tag="f1",
                                        name=f"f1_{c}_{hh}")
                    nc.vector.tensor_tensor(
                        f1[:], th[:, :, 0:64], th[:, :, 64:128], op=ALU.add)
                    f2 = fold_pool.tile([128, HP, 32], bf16, tag="f2",
                                        name=f"f2_{c}_{hh}")
                    nc.vector.tensor_tensor(
                        f2[:], f1[:, :, 0:32], f1[:, :, 32:64], op=ALU.add)
                    f3 = fold_pool.tile([128, HP, 16], bf16, tag="f3",
                                        name=f"f3_{c}_{hh}")
                    nc.vector.tensor_tensor(
                        f3[:], f2[:, :, 0:16], f2[:, :, 16:32], op=ALU.add)
                    nc.vector.reduce_sum(
                        r_sb[:, hh * HP : (hh + 1) * HP], f3[:], axis=AX.X)
                return r_sb

            pending = load_pair0_quarters()
            for c in range(PAIRS):
                parts = pending
                if c + 1 < PAIRS:
                    pending = load_pair(c + 1)
                r_sb = emit_folds(c, parts)
                # node_T[h, (b_lo, n)] via TensorE transpose (f32 in, bf16 out)
                rt_ps = ps_t.tile([128, 128], f32, tag="rt")
                nc.tensor.transpose(rt_ps[:], r_sb[:], i128_sb[:])
                rt_sb = red_pool.tile([128, 128], bf16, tag="rt_sb")
                nc.scalar.activation(rt_sb[:], rt_ps[:], AF.Copy)

                for b_lo in range(2):
                    b = 2 * c + b_lo
                    nodeb = rt_sb[:, 64 * b_lo : 64 * b_lo + 64]
                    w3_ps = ps_w3.tile([8, 512], f32, tag="w3")

                    def h2_stage(chunk, h2_ps):
                        # h2 evac + W3, emitted one chunk late: when this
                        # reaches ACT's FIFO head its W2 matmul finished a
                        # full stage ago -> no cross-engine ping-pong stall.
                        h2_sb = h2_pool.tile([64, 512], bf16, tag="h2",
                                             name=f"h2_{b}_{chunk}")
                        nc.scalar.activation(
                            h2_sb[:], h2_ps[:], AF.Relu,
                            bias=0.0 if NOBIAS else b2_sb[:])
                        nc.tensor.matmul(
                            w3_ps[:],
                            w3_sb[:, 8 * chunk : 8 * chunk + 8],
                            h2_sb[:],
                            start=(chunk == 0), stop=(chunk == NCH - 1))

                    prev = None
                    for chunk in range(NCH):
                        x_ps = ps_x.tile([128, 512], f32, tag="xps")
                        rhs_i = (
                            nodeb[:, 8 * chunk : 8 * chunk + 8]
                            .unsqueeze(2)
                            .broadcast_to((128, 8, 64)))
                        rhs_j = nodeb.unsqueeze(1).broadcast_to((128, 8, 64))
                        nc.tensor.matmul(
                            x_ps[:], wi_sb[:], rhs_i, start=True, stop=False)
                        nc.tensor.matmul(
                            x_ps[:], wj_sb[:], rhs_j, start=False, stop=True)
                        x_sb = x_pool.tile([128, 512], bf16, tag="x")
                        nc.scalar.activation(
                            x_sb[:], x_ps[:], AF.Relu,
                            bias=0.0 if NOBIAS else b1_sb[:])
                        h2_ps = ps_h2.tile([64, 512], f32, tag="h2ps")
                        nc.tensor.matmul(
                            h2_ps[:], w2_sb[:], x_sb[:], start=True, stop=True)
                        if prev is not None:
                            h2_stage(*prev)
                        prev = (chunk, h2_ps)
                    h2_stage(*prev)
                    # F_flat -> F (64, 64): same linearized element order
                    ff_sb = ff_pool.tile([8, 512], f32, tag="ff")
                    if NOBIAS:
                        nc.scalar.activation(ff_sb[:], w3_ps[:], AF.Copy)
                    else:
                        nc.scalar.activation(ff_sb[:], w3_ps[:], AF.Identity,
                                             bias=b3_sb[:])
                    f_sb = ep_pool.tile([N, N], f32, tag="f")
                    nc.sync.dma_start(f_sb[:], ff_sb[:])
                    ft_ps = ps_ft.tile([N, N], f32, tag="ft")
                    nc.tensor.transpose(ft_ps[:], f_sb[:], i64_sb[:, :64])
                    # out = (relu(G + F + F^T) + I) / (rowsum + 1e-8)
                    t1 = ep_pool.tile([N, N], f32, tag="t1")
                    nc.vector.tensor_tensor(t1[:], f_sb[:], ft_ps[:], op=ALU.add)
                    t2 = ep_pool.tile([N, N], f32, tag="t2")
                    nc.vector.tensor_tensor(t2[:], t1[:], g_sb[:], op=ALU.add)
                    sp = ep_pool.tile([N, N], f32, tag="sp")
                    nc.vector.tensor_scalar(
                        sp[:], t2[:], scalar1=0.0, scalar2=None, op0=ALU.max)
                    spi = ep_pool.tile([N, N], f32, tag="spi")
                    nc.vector.tensor_tensor(spi[:], sp[:], i64_sb[:], op=ALU.add)
                    rs = ep_pool.tile([N, 1], f32, tag="rs")
                    nc.vector.reduce_sum(rs[:], spi[:], axis=AX.X)
                    rb = ep_pool.tile([N, 1], f32, tag="rb")
                    nc.vector.tensor_scalar(
                        rb[:], rs[:], scalar1=1e-8, scalar2=None, op0=ALU.add)
                    rec = ep_pool.tile([N, 1], f32, tag="rec")
                    nc.vector.reciprocal(rec[:], rb[:])
                    o_sb = ep_pool.tile([N, N], f32, tag="o")
                    nc.vector.tensor_scalar(
                        o_sb[:], spi[:], scalar1=rec[:], scalar2=None,
                        op0=ALU.mult)
                    nc.sync.dma_start(out_ext[b], o_sb[:])
    return nc


def _get_nc():
    key = ('nc', _CACHE.get('cfg_mono', False), _CACHE.get('cfg_nobias', False))
    if key not in _CACHE:
        _CACHE[key] = build_nc()
    return _CACHE[key]


def kernel(**inputs):
    import ml_dtypes

    from concourse.bass_utils import run_bass_kernel_spmd

    _install_wait_splitter()

    tf = np.asarray(inputs["temporal_features"], dtype=np.float32)
    ap = np.asarray(inputs["adj_param"], dtype=np.float32)
    W1 = np.asarray(inputs["W1"], dtype=np.float32)
    b1 = np.asarray(inputs["b1"], dtype=np.float32)
    W2 = np.asarray(inputs["W2"], dtype=np.float32)
    b2 = np.asarray(inputs["b2"], dtype=np.float32)
    W3 = np.asarray(inputs["W3"], dtype=np.float32)
    b3 = np.asarray(inputs["b3"], dtype=np.float32)

    bf = ml_dtypes.bfloat16
    Wi = np.ascontiguousarray((W1[:H] / T).astype(bf))
    Wj = np.ascontiguousarray((W1[H:] / T).astype(bf))
    b1c = b1.reshape(H, 1)
    b2c = b2.reshape(H // 2, 1)
    # Per chunk, an (H//2, 8) one-hot-column weight routing the chunk's
    # scalar output to PSUM partition `chunk` (0.25 sym factor folded in).
    W3blk = np.zeros((H // 2, NCH, 8), np.float32)
    for chunk in range(NCH):
        W3blk[:, chunk, chunk] = 0.25 * W3[:, 0]
    W3blk = np.ascontiguousarray(W3blk.reshape(H // 2, 8 * NCH).astype(bf))
    b3c = np.full((8, 1), 0.25 * float(b3[0]), np.float32)
    G = np.ascontiguousarray(0.25 * (ap + ap.T))
    I64np = np.eye(N, dtype=np.float32)
    I128np = np.eye(H, dtype=np.float32)

    shared = {
        "Wi": Wi, "Wj": Wj, "b1c": b1c, "W2": np.ascontiguousarray(W2.astype(bf)),
        "b2c": b2c, "W3blk": W3blk, "b3c": b3c, "G": G, "I64": I64np,
        "I128": I128np,
    }
    in_maps = [
        {"tf": np.ascontiguousarray(tf[i * B_LOC : (i + 1) * B_LOC]), **shared}
        for i in range(NCORES)
    ]

    _CACHE['cfg_nobias'] = bool(
        not b1.any() and not b2.any() and not b3.any())
    nc = _get_nc()
    res = run_bass_kernel_spmd(nc, in_maps, core_ids=list(range(NCORES)),
                               **_CACHE.get('run_kwargs', {}))
    _CACHE['last_result'] = res
    out = np.concatenate([res.results[i]["out"] for i in range(NCORES)], axis=0)
    return np.ascontiguousarray(out.astype(np.float32))



# revision 24
# speedup vs baseline: 1.0764x; 1.0764x over previous
"""AdaptiveGraphLearning forward on 8 Trainium2 NeuronCores.

Data-parallel over batch B=64: each core processes 8 batches; the (N,N)
adjacency parameter and tiny edge-MLP weights are replicated (the forward
pass needs no collectives).

Per-core dataflow (8 batches as 4 pairs):
  - HWDGE-DMA two batches of temporal_features per transfer as
    (128, 8192) f32 tiles: partition p=(b_lo,n), free=(h,t) -- 32-64KB
    contiguous DRAM per partition for near-peak HBM bandwidth. Pair 0 is
    split into 2MB quarters spread over both HWDGE rings (SP + ACT) to
    shorten the pipeline-fill latency; later pairs ride the SP ring as
    4MB halves while ACT is busy with evacuations.
  - Sum over t: one f32->bf16 VectorE fold, then bf16 folds in 2x mode
    (t:128->64->32->16) and a short 1x reduce -> R (128=(b_lo,n), 128=h).
  - TensorE transpose -> node_T bf16 (h on partitions). 1/T folded into
    the W1 halves host-side.
  - Edge MLP entirely in bf16 matmuls (1 cycle/row):
      X_pre = Wi.T @ node_T[:,i(bcast)] + Wj.T @ node_T[:,j(bcast)] (PSUM)
      X  = relu(X_pre + b1)   ScalarE evacuation -> bf16 SBUF
      h2 = relu(W2.T X + b2)  evacuations mostly ScalarE, 1-in-8 VectorE
      F  = W3blk.T h2         one-hot block weights accumulate (8,512)
  - F(8,512) -> F(64,64) via SBUF->SBUF DMA (identical linearized element
    order), F^T on TensorE, then
      out = (relu(G + F + F^T) + I) / row-sum
    with G = 0.25*(ap+ap^T) precomputed host-side and the 0.25
    symmetrization factor folded into W3.

Harness notes: walrus in this container accepts a single semaphore wait
per instruction, so a BIR-level pass splits Tile's multi-wait
instructions onto EventSemaphore carriers; the Tile kernel-tail drain
gets the same treatment at build time.
"""
import sys

sys.path.insert(0, '/opt/trn_rl_repo')

import numpy as np

B, N, H, T = 64, 64, 128, 128
NCORES = 8
B_LOC = B // NCORES      # 8 batches per core
PAIRS = B_LOC // 2       # 4 batch pairs per core
NCH = N // 8             # 8 i-chunks per batch (8 i x 64 j = 512 wide)

_CACHE = {}


def _install_wait_splitter():
    """walrus's per-instruction sync structs hold a single semaphore wait;
    Tile can emit several on one instruction. Split extras onto preceding
    single-wait Drain instructions at the BIR-JSON level."""
    if _CACHE.get('wait_splitter'):
        return
    import json

    import concourse.bass2jax as bass2jax

    orig = bass2jax.compile_bir_kernel

    def split_waits_in_bir(bir_bytes):
        d = json.loads(bir_bytes)
        n_new = [0]
        for fn in d.get("functions", []):
            for bb in fn.get("blocks", []):
                out = []
                for ins in bb.get("instructions", []):
                    si = ins.get("sync_info") or {}
                    waits = si.get("on_wait") or []
                    if len(waits) > 1:
                        for w in waits[:-1]:
                            n_new[0] += 1
                            out.append({
                                "engine": ins["engine"],
                                "ins": [],
                                "name": f"IWS-{n_new[0]}",
                                "opcode": "EventSemaphore",
                                "outs": [],
                                "sync_info": {"on_update": [], "on_wait": [w]},
                            })
                        si["on_wait"] = [waits[-1]]
                    out.append(ins)
                bb["instructions"] = out
        return json.dumps(d).encode()

    def wrapper(ant_bir_str, *a, **kw):
        return orig(split_waits_in_bir(ant_bir_str), *a, **kw)

    bass2jax.compile_bir_kernel = wrapper
    _CACHE['wait_splitter'] = True


def _split_drain_tile_context(tile_mod, mybir, nc):
    """TileContext whose kernel-tail drain splits its semaphore waits across
    sequential Drain instructions (walrus CTRL insts accept one wait)."""
    from concourse.tile import ScopedClock

    class SplitDrainTileContext(tile_mod.TileContext):
        def _drain_and_barrier(self, tick_clock, wait_clock):
            drain_inst = self.nc.sync.drain()
            wait_clock.add_sem_waits(
                drain_inst.ins, ScopedClock({None: tick_clock.global_clock})
            )
            waits = list(drain_inst.ins.sync_info.on_wait)
            if len(waits) > 1:
                drain_inst.ins.sync_info = mybir.SyncInfo(
                    on_wait=waits[:1],
                    on_update=list(drain_inst.ins.sync_info.on_update),
                )
                for i in range(1, len(waits)):
                    extra = self.nc.sync.drain()
                    extra.ins.sync_info = mybir.SyncInfo(
                        on_wait=waits[i : i + 1], on_update=[]
                    )
            self.nc.all_engine_barrier()
            assert self.sems is not None
            popped = self.nc._tile_sem_poison_stack.pop()
            assert popped is self._sem_poison
            self.nc.clear_and_free_semaphores(list(self.sems.allocated().values()))
            self.nc.all_engine_barrier()

    return SplitDrainTileContext(nc)


def build_nc():
    import concourse.bass as bass
    import concourse.tile as tile
    from concourse import mybir
    from contextlib import ExitStack

    f32 = mybir.dt.float32
    bf16 = mybir.dt.bfloat16
    AF = mybir.ActivationFunctionType
    ALU = mybir.AluOpType
    AX = mybir.AxisListType

    nc = bass.Bass()
    tf = nc.declare_dram_parameter("tf", [B_LOC, N, H, T], f32, isOutput=False)
    Wi = nc.declare_dram_parameter("Wi", [H, H], bf16, isOutput=False)
    Wj = nc.declare_dram_parameter("Wj", [H, H], bf16, isOutput=False)
    b1c = nc.declare_dram_parameter("b1c", [H, 1], f32, isOutput=False)
    W2 = nc.declare_dram_parameter("W2", [H, H // 2], bf16, isOutput=False)
    b2c = nc.declare_dram_parameter("b2c", [H // 2, 1], f32, isOutput=False)
    W3blk = nc.declare_dram_parameter("W3blk", [H // 2, 8 * NCH], bf16, isOutput=False)
    b3c = nc.declare_dram_parameter("b3c", [8, 1], f32, isOutput=False)
    G = nc.declare_dram_parameter("G", [N, N], f32, isOutput=False)
    I64 = nc.declare_dram_parameter("I64", [N, N], f32, isOutput=False)
    I128 = nc.declare_dram_parameter("I128", [H, H], f32, isOutput=False)
    out_ext = nc.declare_dram_parameter("out", [B_LOC, N, N], f32, isOutput=True)

    MONO = _CACHE.get('cfg_mono', False)
    NOBIAS = _CACHE.get('cfg_nobias', False)

    with _split_drain_tile_context(tile, mybir, nc) as tc, ExitStack() as ctx:
        consts = ctx.enter_context(tc.tile_pool(name="consts", bufs=1))
        tf_pool = ctx.enter_context(
            tc.tile_pool(name="tf", bufs=2 if MONO else 4))
        fold_pool = ctx.enter_context(tc.tile_pool(name="fold", bufs=2))
        red_pool = ctx.enter_context(tc.tile_pool(name="red", bufs=2))
        x_pool = ctx.enter_context(tc.tile_pool(name="x", bufs=4))
        h2_pool = ctx.enter_context(tc.tile_pool(name="h2", bufs=4))
        ff_pool = ctx.enter_context(tc.tile_pool(name="ff", bufs=2))
        ep_pool = ctx.enter_context(tc.tile_pool(name="ep", bufs=2))
        ps_x = ctx.enter_context(tc.tile_pool(name="ps_x", bufs=2, space="PSUM"))
        ps_h2 = ctx.enter_context(tc.tile_pool(name="ps_h2", bufs=2, space="PSUM"))
        ps_w3 = ctx.enter_context(tc.tile_pool(name="ps_w3", bufs=2, space="PSUM"))
        ps_t = ctx.enter_context(tc.tile_pool(name="ps_t", bufs=1, space="PSUM"))
        ps_ft = ctx.enter_context(tc.tile_pool(name="ps_ft", bufs=1, space="PSUM"))
        if True:
            wi_sb = consts.tile([H, H], bf16)
            nc.scalar.dma_start(wi_sb[:], Wi[:])
            wj_sb = consts.tile([H, H], bf16)
            nc.scalar.dma_start(wj_sb[:], Wj[:])
            w2_sb = consts.tile([H, H // 2], bf16)
            nc.scalar.dma_start(w2_sb[:], W2[:])
            w3_sb = consts.tile([H // 2, 8 * NCH], bf16)
            nc.scalar.dma_start(w3_sb[:], W3blk[:])
            b1_sb = consts.tile([H, 1], f32)
            nc.scalar.dma_start(b1_sb[:], b1c[:])
            b2_sb = consts.tile([H // 2, 1], f32)
            nc.scalar.dma_start(b2_sb[:], b2c[:])
            b3_sb = consts.tile([8, 1], f32)
            nc.scalar.dma_start(b3_sb[:], b3c[:])
            g_sb = consts.tile([N, N], f32)
            nc.scalar.dma_start(g_sb[:], G[:])
            i64_sb = consts.tile([N, N], f32)
            nc.scalar.dma_start(i64_sb[:], I64[:])
            i128_sb = consts.tile([H, H], f32)
            nc.scalar.dma_start(i128_sb[:], I128[:])

            HQ = H // 2

            def load_pair(c):
                if MONO:
                    # one monolithic 8MB transfer: single-queue DMA reaches
                    # ~425 GB/s only for large transfers (4MB: ~334)
                    tft = tf_pool.tile([128, H, T], f32, name=f"tft{c}",
                                       tag="tft")
                    nc.sync.dma_start(tft[:], tf[2 * c : 2 * c + 2, :, :, :])
                    return [tft[:, 0:HQ, :], tft[:, HQ:H, :]]
                halves = []
                for hh in range(2):
                    tft = tf_pool.tile([128, HQ, T], f32, name=f"tft{c}_{hh}",
                                       tag="tft")
                    # For the first two pairs, the odd halves ride the ACT
                    # HWDGE ring (its trigger sits at the head of ACT's
                    # still-idle queue) -> both rings run concurrently during
                    # the pipeline-fill window, ~2x load bandwidth.
                    eng = nc.scalar if (c < 2 and hh == 1) else nc.sync
                    eng.dma_start(
                        tft[:],
                        tf[2 * c : 2 * c + 2, :, hh * HQ : (hh + 1) * HQ, :])
                    halves.append(tft[:])
                return halves

            def load_pair0_quarters():
                # Pair 0 in 2MB quarters across both rings with a dedicated
                # pool tag: the first fold starts right after the first
                # quarter lands, and later pairs' half-tiles don't contend
                # for these slots.
                qs = []
                HF = H // 4
                for q in range(4):
                    tft = tf_pool.tile([128, HF, T], f32, name=f"tfq{q}",
                                       tag="tft")
                    eng = nc.sync if q % 2 == 0 else nc.scalar
                    eng.dma_start(
                        tft[:], tf[0:2, :, q * HF : (q + 1) * HF, :])
                    qs.append(tft[:])
                return qs

            def emit_folds(c, parts):
                # Sum over T: R[p=(b_lo,n), h] = sum_t tf[2c+b_lo, n, h, t].
                # Emitted BEFORE the previous pair's MLP so these sit ahead
                # of the wait-gated epilogue ops in the DVE FIFO and start
                # the moment their data lands.
                r_sb = red_pool.tile([128, H], f32, tag="r", name=f"r{c}")
                npc = len(parts)
                HP = H // npc
                for hh in range(npc):
                    th = parts[hh]
                    f1 = fold_pool.tile([128, HP, 64], bf16, tag="f1",
                                        name=f"f1_{c}_{hh}")
                    nc.vector.tensor_tensor(
                        f1[:], th[:, :, 0:64], th[:, :, 64:128], op=ALU.add)
                    f2 = fold_pool.tile([128, HP, 32], bf16, tag="f2",
                                        name=f"f2_{c}_{hh}")
                    nc.vector.tensor_tensor(
                        f2[:], f1[:, :, 0:32], f1[:, :, 32:64], op=ALU.add)
                    f3 = fold_pool.tile([128, HP, 16], bf16, tag="f3",
                                        name=f"f3_{c}_{hh}")
                    nc.vector.tensor_tensor(
                        f3[:], f2[:, :, 0:16], f2[:, :, 16:32], op=ALU.add)
                    nc.vector.reduce_sum(
                        r_sb[:, hh * HP : (hh + 1) * HP], f3[:], axis=AX.X)
                return r_sb

            pending = load_pair0_quarters()
            for c in range(PAIRS):
                parts = pending
                if c + 1 < PAIRS:
                    pending = load_pair(c + 1)
                r_sb = emit_folds(c, parts)
                # node_T[h, (b_lo, n)] via TensorE transpose (f32 in, bf16 out)
                rt_ps = ps_t.tile([128, 128], f32, tag="rt")
                nc.tensor.transpose(rt_ps[:], r_sb[:], i128_sb[:])
                rt_sb = red_pool.tile([128, 128], bf16, tag="rt_sb")
                nc.scalar.activation(rt_sb[:], rt_ps[:], AF.Copy)

                for b_lo in range(2):
                    b = 2 * c + b_lo
                    nodeb = rt_sb[:, 64 * b_lo : 64 * b_lo + 64]
                    w3_ps = ps_w3.tile([8, 512], f32, tag="w3")

                    def h2_stage(chunk, h2_ps):
                        # h2 evac + W3, emitted one chunk late: when this
                        # reaches ACT's FIFO head its W2 matmul finished a
                        # full stage ago -> no cross-engine ping-pong stall.
                        h2_sb = h2_pool.tile([64, 512], bf16, tag="h2",
                                             name=f"h2_{b}_{chunk}")
                        nc.scalar.activation(
                            h2_sb[:], h2_ps[:], AF.Relu,
                            bias=0.0 if NOBIAS else b2_sb[:])
                        nc.tensor.matmul(
                            w3_ps[:],
                            w3_sb[:, 8 * chunk : 8 * chunk + 8],
                            h2_sb[:],
                            start=(chunk == 0), stop=(chunk == NCH - 1))

                    prev = None
                    for chunk in range(NCH):
                        x_ps = ps_x.tile([128, 512], f32, tag="xps")
                        rhs_i = (
                            nodeb[:, 8 * chunk : 8 * chunk + 8]
                            .unsqueeze(2)
                            .broadcast_to((128, 8, 64)))
                        rhs_j = nodeb.unsqueeze(1).broadcast_to((128, 8, 64))
                        nc.tensor.matmul(
                            x_ps[:], wi_sb[:], rhs_i, start=True, stop=False)
                        nc.tensor.matmul(
                            x_ps[:], wj_sb[:], rhs_j, start=False, stop=True)
                        x_sb = x_pool.tile([128, 512], bf16, tag="x")
                        nc.scalar.activation(
                            x_sb[:], x_ps[:], AF.Relu,
                            bias=0.0 if NOBIAS else b1_sb[:])
                        h2_ps = ps_h2.tile([64, 512], f32, tag="h2ps")
                        nc.tensor.matmul(
                            h2_ps[:], w2_sb[:], x_sb[:], start=True, stop=True)
                        if prev is not None:
                            h2_stage(*prev)
                        prev = (chunk, h2_ps)
                    h2_stage(*prev)
                    # F_flat -> F (64, 64): same linearized element order
                    ff_sb = ff_pool.tile([8, 512], f32, tag="ff")
                    if NOBIAS:
                        nc.scalar.activation(ff_sb[:], w3_ps[:], AF.Copy)
                    else:
                        nc.scalar.activation(ff_sb[:], w3_ps[:], AF.Identity,
                                             bias=b3_sb[:])
                    f_sb = ep_pool.tile([N, N], f32, tag="f")
                    nc.sync.dma_start(f_sb[:], ff_sb[:])
                    ft_ps = ps_ft.tile([N, N], f32, tag="ft")
                    nc.tensor.transpose(ft_ps[:], f_sb[:], i64_sb[:, :64])
                    # out = (relu(G + F + F^T) + I) / (rowsum + 1e-8)
                    t1 = ep_pool.tile([N, N], f32, tag="t1")
                    nc.vector.tensor_tensor(t1[:], f_sb[:], ft_ps[:], op=ALU.add)
                    t2 = ep_pool.tile([N, N], f32, tag="t2")
                    nc.vector.tensor_tensor(t2[:], t1[:], g_sb[:], op=ALU.add)
                    sp = ep_pool.tile([N, N], f32, tag="sp")
                    nc.vector.tensor_scalar(
                        sp[:], t2[:], scalar1=0.0, scalar2=None, op0=ALU.max)
                    spi = ep_pool.tile([N, N], f32, tag="spi")
                    nc.vector.tensor_tensor(spi[:], sp[:], i64_sb[:], op=ALU.add)
                    rs = ep_pool.tile([N, 1], f32, tag="rs")
                    nc.vector.reduce_sum(rs[:], spi[:], axis=AX.X)
                    rb = ep_pool.tile([N, 1], f32, tag="rb")
                    nc.vector.tensor_scalar(
                        rb[:], rs[:], scalar1=1e-8, scalar2=None, op0=ALU.add)
                    rec = ep_pool.tile([N, 1], f32, tag="rec")
                    nc.vector.reciprocal(rec[:], rb[:])
                    o_sb = ep_pool.tile([N, N], f32, tag="o")
                    nc.vector.tensor_scalar(
                        o_sb[:], spi[:], scalar1=rec[:], scalar2=None,
                        op0=ALU.mult)
                    nc.sync.dma_start(out_ext[b], o_sb[:])
    return nc


def _get_nc():
    key = ('nc', _CACHE.get('cfg_mono', False), _CACHE.get('cfg_nobias', False))
    if key not in _CACHE:
        _CACHE[key] = build_nc()
    return _CACHE[key]


def kernel(**inputs):
    import ml_dtypes

    from concourse.bass_utils import run_bass_kernel_spmd

    _install_wait_splitter()

    tf = np.asarray(inputs["temporal_features"], dtype=np.float32)
    ap = np.asarray(inputs["adj_param"], dtype=np.float32)
    W1 = np.asarray(inputs["W1"], dtype=np.float32)
    b1 = np.asarray(inputs["b1"], dtype=np.float32)
    W2 = np.asarray(inputs["W2"], dtype=np.float32)
    b2 = np.asarray(inputs["b2"], dtype=np.float32)
    W3 = np.asarray(inputs["W3"], dtype=np.float32)
    b3 = np.asarray(inputs["b3"], dtype=np.float32)

    bf = ml_dtypes.bfloat16
    Wi = np.ascontiguousarray((W1[:H] / T).astype(bf))
    Wj = np.ascontiguousarray((W1[H:] / T).astype(bf))
    b1c = b1.reshape(H, 1)
    b2c = b2.reshape(H // 2, 1)
    # Per chunk, an (H//2, 8) one-hot-column weight routing the chunk's
    # scalar output to PSUM partition `chunk` (0.25 sym factor folded in).
    W3blk = np.zeros((H // 2, NCH, 8), np.float32)
    for chunk in range(NCH):
        W3blk[:, chunk, chunk] = 0.25 * W3[:, 0]
    W3blk = np.ascontiguousarray(W3blk.reshape(H // 2, 8 * NCH).astype(bf))
    b3c = np.full((8, 1), 0.25 * float(b3[0]), np.float32)
    G = np.ascontiguousarray(0.25 * (ap + ap.T))
    I64np = np.eye(N, dtype=np.float32)
    I128np = np.eye(H, dtype=np.float32)

    shared = {
        "Wi": Wi, "Wj": Wj, "b1c": b1c, "W2": np.ascontiguousarray(W2.astype(bf)),
        "b2c": b2c, "W3blk": W3blk, "b3c": b3c, "G": G, "I64": I64np,
        "I128": I128np,
    }
    in_maps = [
        {"tf": np.ascontiguousarray(tf[i * B_LOC : (i + 1) * B_LOC]), **shared}
        for i in range(NCORES)
    ]

    _CACHE['cfg_nobias'] = bool(
        not b1.any() and not b2.any() and not b3.any())
    nc = _get_nc()
    res = run_bass_kernel_spmd(nc, in_maps, core_ids=list(range(NCORES)),
                               **_CACHE.get('run_kwargs', {}))
    _CACHE['last_result'] = res
    out = np.concatenate([res.results[i]["out"] for i in range(NCORES)], axis=0)
    return np.ascontiguousarray(out.astype(np.float32))

